# revision 29
# baseline (speedup 1.0000x reference)
"""Multi-head attention (B=2, S=2048, D=1024, H=16) on 8 Trainium2 NeuronCores.

Sharding: core = b*4 + hg  (b = batch, hg = head-group of 4 heads).
Each core computes, for its batch b and its 4 heads:
    q^T = (Wq_g @ X_q^T + bq_g)      stored [256, 2048]  (head-dim on partitions)
    k^T likewise, v = X_v @ Wv_g^T + bv_g stored [2048, 4, 65] (ones col appended)
    S^T[s_k, s_q] = k^T.T-contraction(d)  per head
    P^T = exp(S^T / 8)                (no max-subtract; scores are O(1) here)
    attnout^T[o, s_q] (+ rowsum row) = [v | 1].T @ P^T   accumulated over s_k
    attnout_norm^T = attnout^T * (1/rowsum)  (per head)
    outT_partial[m, s] = Wo_g^T-contraction(o) @ attnout_norm^T   [1024, 2048] f32
Host gathers: out[b] = sum_g outT_partial.T + bo.

All matmul inputs bf16 (PSUM accumulation f32).
"""

import numpy as np
import ml_dtypes

import concourse.bacc as bacc
import concourse.mybir as mybir
import concourse.tile as tile
from concourse.bass_utils import run_bass_kernel_spmd

BF16 = mybir.dt.bfloat16
F32 = mybir.dt.float32
AF = mybir.ActivationFunctionType
ALU = mybir.AluOpType

B, S, D = 2, 2048, 1024
H = 16
DK = 64
NCORES = 8
HG = 4  # head groups
HPG = 4  # heads per group
GO = HPG * DK  # 256 group output width

_NC = None


def _emit(nc, tc, io):
    xqT, xkT, xvT, wqT, wkT, wvT, woT, bqk, outT = (
        io["xqT"], io["xkT"], io["xvT"], io["wqT"], io["wkT"], io["wvT"],
        io["woT"], io["bqk"], io["outT"],
    )
    NIC = D // 128  # 8 contraction chunks of 128
    NSC = S // 128  # 16 s chunks of 128
    NSQ = S // 512  # 4 s chunks of 512

    with (
        tc.tile_pool(name="wp", bufs=1) as wp,
        tc.tile_pool(name="xp", bufs=1) as xp,
        tc.tile_pool(name="pp", bufs=1) as pp,
        tc.tile_pool(name="pt", bufs=6) as ptp,
        tc.tile_pool(name="rs", bufs=2) as rsp,
        tc.tile_pool(name="rb", bufs=2) as rbp,
        tc.tile_pool(name="st", bufs=2) as stp,
        tc.tile_pool(name="fo", bufs=8) as fop,
        tc.tile_pool(name="psA", bufs=4, space="PSUM") as psA,
        tc.tile_pool(name="psB", bufs=2, space="PSUM") as psB,
        tc.tile_pool(name="dr", bufs=4, space="DRAM") as drp,
    ):
        # ---- load weights + inputs (in consumption order) ----
        bqk_t = wp.tile([128, 4], F32, name="bqk", tag="bqk")
        nc.sync.dma_start(bqk_t[:], bqk[:])
        wk = []
        xk = []
        for i in range(NIC):
            t = wp.tile([128, GO], BF16, name=f"wk{i}", tag=f"wk{i}")
            nc.sync.dma_start(t[:], wkT[128 * i:128 * i + 128, :])
            wk.append(t)
            t = xp.tile([128, S], BF16, name=f"xk{i}", tag=f"xk{i}")
            nc.gpsimd.dma_start(t[:], xkT[128 * i:128 * i + 128, :])
            xk.append(t)
        wq = []
        xq = []
        for i in range(NIC):
            t = wp.tile([128, GO], BF16, name=f"wq{i}", tag=f"wq{i}")
            nc.sync.dma_start(t[:], wqT[128 * i:128 * i + 128, :])
            wq.append(t)
            t = xp.tile([128, S], BF16, name=f"xq{i}", tag=f"xq{i}")
            nc.gpsimd.dma_start(t[:], xqT[128 * i:128 * i + 128, :])
            xq.append(t)
        wv = []
        xv = []
        for i in range(NIC):
            t = wp.tile([128, GO], BF16, name=f"wv{i}", tag=f"wv{i}")
            nc.sync.dma_start(t[:], wvT[128 * i:128 * i + 128, :])
            wv.append(t)
            t = xp.tile([128, S], BF16, name=f"xv{i}", tag=f"xv{i}")
            nc.gpsimd.dma_start(t[:], xvT[128 * i:128 * i + 128, :])
            xv.append(t)
        wv_b = wp.tile([1, GO], BF16, name="wvb", tag="wvb")
        nc.sync.dma_start(wv_b[:], wvT[D:D + 1, :])
        xv_ones = xp.tile([1, S], BF16, name="xvo", tag="xvo")
        nc.sync.dma_start(xv_ones[:], xvT[D:D + 1, :])
        wo = []
        for oc in range(2):
            t = wp.tile([128, D], BF16, name=f"wo{oc}", tag=f"wo{oc}")
            nc.sync.dma_start(t[:], woT[128 * oc:128 * oc + 128, :])
            wo.append(t)

        # ---- k^T / q^T projections (ic outer: accumulate as input DMAs
        # land; 4 PSUM accumulators per oc pass) ----
        def proj_T(out_tiles, w_tiles, x_tiles, bias_col0):
            for oc in range(2):
                accs = [psA.tile([128, 512], F32, name="acc", tag="acc")
                        for _ in range(NSQ)]
                for ic in range(NIC):
                    for sc in range(NSQ):
                        nc.tensor.matmul(
                            accs[sc][:],
                            w_tiles[ic][:, 128 * oc:128 * oc + 128],
                            x_tiles[ic][:, 512 * sc:512 * sc + 512],
                            start=(ic == 0),
                            stop=(ic == NIC - 1),
                        )
                for sc in range(NSQ):
                    nc.vector.tensor_scalar(
                        out_tiles[oc][:, 512 * sc:512 * sc + 512], accs[sc][:],
                        bqk_t[:, bias_col0 + oc:bias_col0 + oc + 1], None,
                        op0=ALU.add,
                    )

        kT = [pp.tile([128, S], BF16, name=f"kT{oc}", tag=f"kT{oc}") for oc in range(2)]
        proj_T(kT, wk, xk, 2)

        # ---- q^T projection ----
        qT = [pp.tile([128, S], BF16, name=f"qT{oc}", tag=f"qT{oc}") for oc in range(2)]
        proj_T(qT, wq, xq, 0)

        # ---- v projection: v[sc] [128, 4, 65] (ones col at 64) ----
        v = [pp.tile([128, HPG, DK + 1], BF16, name=f"v{sc}", tag=f"v{sc}") for sc in range(NSC)]
        for sc in range(NSC):
            nc.vector.memset(v[sc][:, :, DK:DK + 1], 1.0)
        for scg in range(NSC // NSQ):
            accs = [psA.tile([128, 512], F32, name="acc", tag="acc")
                    for _ in range(NSQ)]
            for ic in range(NIC):
                for j in range(NSQ):
                    sc = scg * NSQ + j
                    nc.tensor.matmul(
                        accs[j][:, 0:GO],
                        xv[ic][:, 128 * sc:128 * sc + 128],
                        wv[ic][:],
                        start=(ic == 0),
                        stop=False,
                    )
            for j in range(NSQ):
                sc = scg * NSQ + j
                nc.tensor.matmul(
                    accs[j][:, 0:GO],
                    xv_ones[:, 128 * sc:128 * sc + 128],
                    wv_b[:],
                    start=False,
                    stop=True,
                )
                nc.scalar.copy(
                    v[sc][:, :, 0:DK],
                    accs[j][:, 0:GO].rearrange("p (h d) -> p h d", d=DK),
                )

        # ---- attention (1-step software pipeline: scores/exp run one step
        # ahead of the PV matmuls; psB double-buffered per head-pair) ----
        attnT = [pp.tile([128, S], BF16, name=f"at{oc}", tag=f"at{oc}") for oc in range(2)]
        pvacc = {}  # sqc -> 4 PSUM accum tiles

        def normalize(c):
            # Drain the 4 PSUM accumulators to SBUF first (releases the PSUM
            # tiles for reuse); normalization then runs off the critical
            # path from the SBUF copies.  The last chunk computes 1/rowsum
            # as exp(-ln(x)) on the (then idle) scalar engine instead of the
            # slow DVE reciprocal, shortening the kernel tail.
            atcs = []
            for h in range(HPG):
                atc = rsp.tile([DK + 1, 512], F32, name="atc", tag="atc", bufs=5)
                if c == NSQ - 1:
                    nc.scalar.copy(atc[:], pvacc[c][h][0:DK + 1, :])
                else:
                    nc.vector.tensor_copy(atc[:], pvacc[c][h][0:DK + 1, :])
                atcs.append(atc)
            for h in range(HPG):
                atc = atcs[h]
                rs_ = rsp.tile([DK + 1, 512], F32, name="rs", tag="rs")
                if c == NSQ - 1:
                    ln_ = rsp.tile([DK + 1, 512], F32, name="ln", tag="ln")
                    nc.scalar.activation(ln_[DK:DK + 1, :], atc[DK:DK + 1, :],
                                         AF.Ln)
                    nc.scalar.activation(rs_[DK:DK + 1, :], ln_[DK:DK + 1, :],
                                         AF.Exp, scale=-1.0)
                else:
                    nc.vector.reciprocal(rs_[DK:DK + 1, :], atc[DK:DK + 1, :])
                rd_ = drp.tile([1, 512], F32, name="rd", tag="rd")
                nc.sync.dma_start(rd_[:], rs_[DK:DK + 1, :])
                rb_ = rbp.tile([DK, 512], F32, name="rb", tag="rb")
                nc.gpsimd.dma_start(rb_[:], rd_.to_broadcast([DK, 512]))
                st_ = stp.tile([DK, 512], BF16, name="st", tag="st")
                nc.vector.tensor_mul(st_[:], atc[0:DK, :], rb_[:])
                nc.gpsimd.dma_start(
                    attnT[h // 2][64 * (h % 2):64 * (h % 2) + 64,
                                  512 * c:512 * c + 512],
                    st_[:],
                )

        def emit_fproj(sc):
            # Output projection for s-chunk sc (runs at the very end; F(0..2)
            # overlap the last chunk's normalization chain).  Later chunks
            # drain their PSUM through the scalar engine to split the copy
            # load across both PSUM-capable engines.
            for mc in range(D // 128):
                fac = psA.tile([128, 512], F32, name="acc", tag="acc")
                for oc in range(2):
                    nc.tensor.matmul(
                        fac[:],
                        wo[oc][:, 128 * mc:128 * mc + 128],
                        attnT[oc][:, 512 * sc:512 * sc + 512],
                        start=(oc == 0),
                        stop=(oc == 1),
                    )
                fo_ = fop.tile([128, 512], BF16, name="fo", tag="fo")
                if sc >= 2:
                    nc.scalar.copy(fo_[:], fac[:])
                else:
                    nc.vector.tensor_copy(fo_[:], fac[:])
                eng = nc.sync if mc % 2 == 0 else nc.gpsimd
                eng.dma_start(
                    outT[128 * mc:128 * mc + 128, 512 * sc:512 * sc + 512],
                    fo_[:],
                )

        def emit_pv(prev):
            pTs, c, k = prev
            if k == 0:
                pvacc[c] = [
                    psA.tile([128, 512], F32, name="acc", tag="acc")
                    for _ in range(HPG)
                ]
            for h in range(HPG):
                nc.tensor.matmul(
                    pvacc[c][h][0:DK + 1, :],
                    v[k][:, h, :],
                    pTs[h // 2][:, 512 * (h % 2):512 * (h % 2) + 512],
                    start=(k == 0),
                    stop=(k == NSC - 1),
                )
            if k == NSC - 1:
                normalize(c)

        prev = None
        for sqc in range(NSQ):
            for skc in range(NSC):
                pTs = []
                for hp in range(2):
                    ps_ = psB.tile([128, 1024], F32, name="sc", tag="sc")
                    for sub in range(2):
                        nc.tensor.matmul(
                            ps_[:, 512 * sub:512 * sub + 512],
                            kT[hp][64 * sub:64 * sub + 64,
                                   128 * skc:128 * skc + 128],
                            qT[hp][64 * sub:64 * sub + 64,
                                   512 * sqc:512 * sqc + 512],
                            start=True,
                            stop=True,
                            tile_position=(64 * sub, 0),
                        )
                    pT_ = ptp.tile([128, 1024], BF16, name="pT", tag="pT")
                    nc.scalar.activation(pT_[:], ps_[:], AF.Exp, scale=0.125)
                    pTs.append(pT_)
                if prev is not None:
                    emit_pv(prev)
                prev = (pTs, sqc, skc)
        emit_pv(prev)
        for sc in range(NSQ):
            emit_fproj(sc)


def build_nc():
    nc = bacc.Bacc("TRN2", target_bir_lowering=False, debug=False,
                   num_devices=NCORES)
    io = {
        "xqT": nc.dram_tensor("xqT", [D, S], BF16, kind="ExternalInput").ap(),
        "xkT": nc.dram_tensor("xkT", [D, S], BF16, kind="ExternalInput").ap(),
        "xvT": nc.dram_tensor("xvT", [D + 1, S], BF16, kind="ExternalInput").ap(),
        "wqT": nc.dram_tensor("wqT", [D, GO], BF16, kind="ExternalInput").ap(),
        "wkT": nc.dram_tensor("wkT", [D, GO], BF16, kind="ExternalInput").ap(),
        "wvT": nc.dram_tensor("wvT", [D + 1, GO], BF16, kind="ExternalInput").ap(),
        "woT": nc.dram_tensor("woT", [GO, D], BF16, kind="ExternalInput").ap(),
        "bqk": nc.dram_tensor("bqk", [128, 4], F32, kind="ExternalInput").ap(),
        "outT": nc.dram_tensor("outT", [D, S], BF16, kind="ExternalOutput").ap(),
    }
    with tile.TileContext(nc) as tc:
        _emit(nc, tc, io)
    nc.compile()
    return nc


def get_nc():
    global _NC
    if _NC is None:
        _NC = build_nc()
    return _NC


def shard_inputs(Q, K, V, Wq, bq, Wk, bk, Wv, bv, Wo, bo):
    bf = ml_dtypes.bfloat16
    ones = np.ones((1, S), np.float32)
    in_maps = []
    for core in range(NCORES):
        b, hg = core // HG, core % HG
        rows = slice(GO * hg, GO * hg + GO)
        bq_g, bk_g, bv_g = bq[rows], bk[rows], bv[rows]
        bqk_t = np.stack(
            [bq_g[0:128], bq_g[128:256], bk_g[0:128], bk_g[128:256]], axis=1
        ).astype(np.float32)
        in_maps.append({
            "xqT": np.ascontiguousarray(Q[b].T).astype(bf),
            "xkT": np.ascontiguousarray(K[b].T).astype(bf),
            "xvT": np.concatenate([V[b].T, ones], 0).astype(bf),
            "wqT": np.ascontiguousarray(Wq[rows].T).astype(bf),
            "wkT": np.ascontiguousarray(Wk[rows].T).astype(bf),
            "wvT": np.concatenate([Wv[rows].T, bv_g[None, :]], 0).astype(bf),
            "woT": np.ascontiguousarray(Wo[:, rows].T).astype(bf),
            "bqk": bqk_t,
        })
    return in_maps


def kernel(**inputs):
    args = {k: np.asarray(v) for k, v in inputs.items()}
    nc = get_nc()
    in_maps = shard_inputs(
        args["Q"], args["K"], args["V"], args["Wq"], args["bq"], args["Wk"],
        args["bk"], args["Wv"], args["bv"], args["Wo"], args["bo"],
    )
    res = run_bass_kernel_spmd(nc, in_maps, list(range(NCORES)))
    out = np.zeros((B, S, D), np.float32)
    for core in range(NCORES):
        out[core // HG] += res.results[core]["outT"].astype(np.float32).T
    out += args["bo"].astype(np.float32)
    return out


# revision 30
# speedup vs baseline: 1.0104x; 1.0104x over previous
"""Multi-head attention (B=2, S=2048, D=1024, H=16) on 8 Trainium2 NeuronCores.

Sharding: core = b*4 + hg  (b = batch, hg = head-group of 4 heads).
Each core computes, for its batch b and its 4 heads:
    q^T = (Wq_g @ X_q^T + bq_g)      stored [256, 2048]  (head-dim on partitions)
    k^T likewise, v = X_v @ Wv_g^T + bv_g stored [2048, 4, 65] (ones col appended)
    S^T[s_k, s_q] = k^T.T-contraction(d)  per head
    P^T = exp(S^T / 8)                (no max-subtract; scores are O(1) here)
    attnout^T[o, s_q] (+ rowsum row) = [v | 1].T @ P^T   accumulated over s_k
    attnout_norm^T = attnout^T * (1/rowsum)  (per head)
    outT_partial[m, s] = Wo_g^T-contraction(o) @ attnout_norm^T   [1024, 2048] f32
Host gathers: out[b] = sum_g outT_partial.T + bo.

All matmul inputs bf16 (PSUM accumulation f32).
"""

import numpy as np
import ml_dtypes

import concourse.bacc as bacc
import concourse.mybir as mybir
import concourse.tile as tile
from concourse.bass_utils import run_bass_kernel_spmd

BF16 = mybir.dt.bfloat16
F32 = mybir.dt.float32
AF = mybir.ActivationFunctionType
ALU = mybir.AluOpType

B, S, D = 2, 2048, 1024
H = 16
DK = 64
NCORES = 8
HG = 4  # head groups
HPG = 4  # heads per group
GO = HPG * DK  # 256 group output width

_NC = None


def _emit(nc, tc, io):
    xqT, xkT, xvT, wqT, wkT, wvT, woT, bqk, outT = (
        io["xqT"], io["xkT"], io["xvT"], io["wqT"], io["wkT"], io["wvT"],
        io["woT"], io["bqk"], io["outT"],
    )
    NIC = D // 128  # 8 contraction chunks of 128
    NSC = S // 128  # 16 s chunks of 128
    NSQ = S // 512  # 4 s chunks of 512

    with (
        tc.tile_pool(name="wp", bufs=1) as wp,
        tc.tile_pool(name="xp", bufs=1) as xp,
        tc.tile_pool(name="pp", bufs=1) as pp,
        tc.tile_pool(name="pt", bufs=6) as ptp,
        tc.tile_pool(name="rs", bufs=2) as rsp,
        tc.tile_pool(name="rb", bufs=2) as rbp,
        tc.tile_pool(name="st", bufs=2) as stp,
        tc.tile_pool(name="fo", bufs=8) as fop,
        tc.tile_pool(name="psA", bufs=4, space="PSUM") as psA,
        tc.tile_pool(name="psB", bufs=2, space="PSUM") as psB,
        tc.tile_pool(name="dr", bufs=4, space="DRAM") as drp,
    ):
        # ---- load weights + inputs (in consumption order) ----
        bqk_t = wp.tile([128, 4], F32, name="bqk", tag="bqk")
        nc.sync.dma_start(bqk_t[:], bqk[:])
        wk = []
        xk = []
        for i in range(NIC):
            t = wp.tile([128, GO], BF16, name=f"wk{i}", tag=f"wk{i}")
            nc.sync.dma_start(t[:], wkT[128 * i:128 * i + 128, :])
            wk.append(t)
            t = xp.tile([128, S], BF16, name=f"xk{i}", tag=f"xk{i}")
            nc.gpsimd.dma_start(t[:], xkT[128 * i:128 * i + 128, :])
            xk.append(t)
        wq = []
        xq = []
        for i in range(NIC):
            t = wp.tile([128, GO], BF16, name=f"wq{i}", tag=f"wq{i}")
            nc.sync.dma_start(t[:], wqT[128 * i:128 * i + 128, :])
            wq.append(t)
            t = xp.tile([128, S], BF16, name=f"xq{i}", tag=f"xq{i}")
            nc.gpsimd.dma_start(t[:], xqT[128 * i:128 * i + 128, :])
            xq.append(t)
        wv = []
        xv = []
        for i in range(NIC):
            t = wp.tile([128, GO], BF16, name=f"wv{i}", tag=f"wv{i}")
            nc.sync.dma_start(t[:], wvT[128 * i:128 * i + 128, :])
            wv.append(t)
            t = xp.tile([128, S], BF16, name=f"xv{i}", tag=f"xv{i}")
            nc.gpsimd.dma_start(t[:], xvT[128 * i:128 * i + 128, :])
            xv.append(t)
        wv_b = wp.tile([1, GO], BF16, name="wvb", tag="wvb")
        nc.sync.dma_start(wv_b[:], wvT[D:D + 1, :])
        xv_ones = xp.tile([1, S], BF16, name="xvo", tag="xvo")
        nc.sync.dma_start(xv_ones[:], xvT[D:D + 1, :])
        wo = []
        for oc in range(2):
            t = wp.tile([128, D], BF16, name=f"wo{oc}", tag=f"wo{oc}")
            nc.sync.dma_start(t[:], woT[128 * oc:128 * oc + 128, :])
            wo.append(t)

        # ---- k^T / q^T projections (ic outer: accumulate as input DMAs
        # land; 4 PSUM accumulators per oc pass) ----
        def proj_T(out_tiles, w_tiles, x_tiles, bias_col0):
            for oc in range(2):
                accs = [psA.tile([128, 512], F32, name="acc", tag="acc")
                        for _ in range(NSQ)]
                for ic in range(NIC):
                    for sc in range(NSQ):
                        nc.tensor.matmul(
                            accs[sc][:],
                            w_tiles[ic][:, 128 * oc:128 * oc + 128],
                            x_tiles[ic][:, 512 * sc:512 * sc + 512],
                            start=(ic == 0),
                            stop=(ic == NIC - 1),
                        )
                for sc in range(NSQ):
                    nc.vector.tensor_scalar(
                        out_tiles[oc][:, 512 * sc:512 * sc + 512], accs[sc][:],
                        bqk_t[:, bias_col0 + oc:bias_col0 + oc + 1], None,
                        op0=ALU.add,
                    )

        kT = [pp.tile([128, S], BF16, name=f"kT{oc}", tag=f"kT{oc}") for oc in range(2)]
        proj_T(kT, wk, xk, 2)

        # ---- q^T projection ----
        qT = [pp.tile([128, S], BF16, name=f"qT{oc}", tag=f"qT{oc}") for oc in range(2)]
        proj_T(qT, wq, xq, 0)

        # ---- v projection: v[sc] [128, 4, 65] (ones col at 64) ----
        v = [pp.tile([128, HPG, DK + 1], BF16, name=f"v{sc}", tag=f"v{sc}") for sc in range(NSC)]
        for sc in range(NSC):
            nc.vector.memset(v[sc][:, :, DK:DK + 1], 1.0)
        for scg in range(NSC // NSQ):
            accs = [psA.tile([128, 512], F32, name="acc", tag="acc")
                    for _ in range(NSQ)]
            for ic in range(NIC):
                for j in range(NSQ):
                    sc = scg * NSQ + j
                    nc.tensor.matmul(
                        accs[j][:, 0:GO],
                        xv[ic][:, 128 * sc:128 * sc + 128],
                        wv[ic][:],
                        start=(ic == 0),
                        stop=False,
                    )
            for j in range(NSQ):
                sc = scg * NSQ + j
                nc.tensor.matmul(
                    accs[j][:, 0:GO],
                    xv_ones[:, 128 * sc:128 * sc + 128],
                    wv_b[:],
                    start=False,
                    stop=True,
                )
                nc.scalar.copy(
                    v[sc][:, :, 0:DK],
                    accs[j][:, 0:GO].rearrange("p (h d) -> p h d", d=DK),
                )

        # ---- attention (1-step software pipeline: scores/exp run one step
        # ahead of the PV matmuls; psB double-buffered per head-pair) ----
        attnT = [pp.tile([128, S], BF16, name=f"at{oc}", tag=f"at{oc}") for oc in range(2)]
        pvacc = {}  # sqc -> 4 PSUM accum tiles

        def normalize(c):
            # Drain the 4 PSUM accumulators to SBUF first (releases the PSUM
            # tiles for reuse); normalization then runs off the critical
            # path from the SBUF copies.  The last chunk computes 1/rowsum
            # as exp(-ln(x)) on the (then idle) scalar engine instead of the
            # slow DVE reciprocal, shortening the kernel tail.
            atcs = []
            for h in range(HPG):
                atc = rsp.tile([DK + 1, 512], F32, name="atc", tag="atc", bufs=5)
                nc.vector.tensor_copy(atc[:], pvacc[c][h][0:DK + 1, :])
                atcs.append(atc)
            for h in range(HPG):
                atc = atcs[h]
                rs_ = rsp.tile([DK + 1, 512], F32, name="rs", tag="rs")
                if c == NSQ - 1:
                    ln_ = rsp.tile([DK + 1, 512], F32, name="ln", tag="ln")
                    nc.scalar.activation(ln_[DK:DK + 1, :], atc[DK:DK + 1, :],
                                         AF.Ln)
                    nc.scalar.activation(rs_[DK:DK + 1, :], ln_[DK:DK + 1, :],
                                         AF.Exp, scale=-1.0)
                else:
                    nc.vector.reciprocal(rs_[DK:DK + 1, :], atc[DK:DK + 1, :])
                rd_ = drp.tile([1, 512], F32, name="rd", tag="rd")
                nc.sync.dma_start(rd_[:], rs_[DK:DK + 1, :])
                rb_ = rbp.tile([DK, 512], F32, name="rb", tag="rb")
                nc.gpsimd.dma_start(rb_[:], rd_.to_broadcast([DK, 512]))
                st_ = stp.tile([DK, 512], BF16, name="st", tag="st")
                nc.vector.tensor_mul(st_[:], atc[0:DK, :], rb_[:])
                nc.gpsimd.dma_start(
                    attnT[h // 2][64 * (h % 2):64 * (h % 2) + 64,
                                  512 * c:512 * c + 512],
                    st_[:],
                )

        def emit_fproj(sc):
            # Output projection for s-chunk sc (runs at the very end; F(0..2)
            # overlap the last chunk's normalization chain).  Later chunks
            # drain their PSUM through the scalar engine to split the copy
            # load across both PSUM-capable engines.
            for mc in range(D // 128):
                fac = psA.tile([128, 512], F32, name="acc", tag="acc")
                for oc in range(2):
                    nc.tensor.matmul(
                        fac[:],
                        wo[oc][:, 128 * mc:128 * mc + 128],
                        attnT[oc][:, 512 * sc:512 * sc + 512],
                        start=(oc == 0),
                        stop=(oc == 1),
                    )
                fo_ = fop.tile([128, 512], BF16, name="fo", tag="fo")
                if sc >= 2:
                    nc.scalar.copy(fo_[:], fac[:])
                else:
                    nc.vector.tensor_copy(fo_[:], fac[:])
                eng = nc.sync if mc % 2 == 0 else nc.gpsimd
                eng.dma_start(
                    outT[128 * mc:128 * mc + 128, 512 * sc:512 * sc + 512],
                    fo_[:],
                )

        def emit_pv(prev):
            pTs, c, k = prev
            if k == 0:
                pvacc[c] = [
                    psA.tile([128, 512], F32, name="acc", tag="acc")
                    for _ in range(HPG)
                ]
            for h in range(HPG):
                nc.tensor.matmul(
                    pvacc[c][h][0:DK + 1, :],
                    v[k][:, h, :],
                    pTs[h // 2][:, 512 * (h % 2):512 * (h % 2) + 512],
                    start=(k == 0),
                    stop=(k == NSC - 1),
                )
            if k == NSC - 1:
                normalize(c)

        prev = None
        for sqc in range(NSQ):
            for skc in range(NSC):
                pTs = []
                for hp in range(2):
                    ps_ = psB.tile([128, 1024], F32, name="sc", tag="sc")
                    for sub in range(2):
                        nc.tensor.matmul(
                            ps_[:, 512 * sub:512 * sub + 512],
                            kT[hp][64 * sub:64 * sub + 64,
                                   128 * skc:128 * skc + 128],
                            qT[hp][64 * sub:64 * sub + 64,
                                   512 * sqc:512 * sqc + 512],
                            start=True,
                            stop=True,
                            tile_position=(64 * sub, 0),
                        )
                    pT_ = ptp.tile([128, 1024], BF16, name="pT", tag="pT")
                    nc.scalar.activation(pT_[:], ps_[:], AF.Exp, scale=0.125)
                    pTs.append(pT_)
                if prev is not None:
                    emit_pv(prev)
                prev = (pTs, sqc, skc)
        emit_pv(prev)
        for sc in range(NSQ):
            emit_fproj(sc)


def build_nc():
    nc = bacc.Bacc("TRN2", target_bir_lowering=False, debug=False,
                   num_devices=NCORES)
    io = {
        "xqT": nc.dram_tensor("xqT", [D, S], BF16, kind="ExternalInput").ap(),
        "xkT": nc.dram_tensor("xkT", [D, S], BF16, kind="ExternalInput").ap(),
        "xvT": nc.dram_tensor("xvT", [D + 1, S], BF16, kind="ExternalInput").ap(),
        "wqT": nc.dram_tensor("wqT", [D, GO], BF16, kind="ExternalInput").ap(),
        "wkT": nc.dram_tensor("wkT", [D, GO], BF16, kind="ExternalInput").ap(),
        "wvT": nc.dram_tensor("wvT", [D + 1, GO], BF16, kind="ExternalInput").ap(),
        "woT": nc.dram_tensor("woT", [GO, D], BF16, kind="ExternalInput").ap(),
        "bqk": nc.dram_tensor("bqk", [128, 4], F32, kind="ExternalInput").ap(),
        "outT": nc.dram_tensor("outT", [D, S], BF16, kind="ExternalOutput").ap(),
    }
    with tile.TileContext(nc) as tc:
        _emit(nc, tc, io)
    nc.compile()
    return nc


def get_nc():
    global _NC
    if _NC is None:
        _NC = build_nc()
    return _NC


def shard_inputs(Q, K, V, Wq, bq, Wk, bk, Wv, bv, Wo, bo):
    bf = ml_dtypes.bfloat16
    ones = np.ones((1, S), np.float32)
    in_maps = []
    for core in range(NCORES):
        b, hg = core // HG, core % HG
        rows = slice(GO * hg, GO * hg + GO)
        bq_g, bk_g, bv_g = bq[rows], bk[rows], bv[rows]
        bqk_t = np.stack(
            [bq_g[0:128], bq_g[128:256], bk_g[0:128], bk_g[128:256]], axis=1
        ).astype(np.float32)
        in_maps.append({
            "xqT": np.ascontiguousarray(Q[b].T).astype(bf),
            "xkT": np.ascontiguousarray(K[b].T).astype(bf),
            "xvT": np.concatenate([V[b].T, ones], 0).astype(bf),
            "wqT": np.ascontiguousarray(Wq[rows].T).astype(bf),
            "wkT": np.ascontiguousarray(Wk[rows].T).astype(bf),
            "wvT": np.concatenate([Wv[rows].T, bv_g[None, :]], 0).astype(bf),
            "woT": np.ascontiguousarray(Wo[:, rows].T).astype(bf),
            "bqk": bqk_t,
        })
    return in_maps


def kernel(**inputs):
    args = {k: np.asarray(v) for k, v in inputs.items()}
    nc = get_nc()
    in_maps = shard_inputs(
        args["Q"], args["K"], args["V"], args["Wq"], args["bq"], args["Wk"],
        args["bk"], args["Wv"], args["bv"], args["Wo"], args["bo"],
    )
    res = run_bass_kernel_spmd(nc, in_maps, list(range(NCORES)))
    out = np.zeros((B, S, D), np.float32)
    for core in range(NCORES):
        out[core // HG] += res.results[core]["outT"].astype(np.float32).T
    out += args["bo"].astype(np.float32)
    return out


# revision 31
# speedup vs baseline: 1.0207x; 1.0102x over previous
"""Multi-head attention (B=2, S=2048, D=1024, H=16) on 8 Trainium2 NeuronCores.

Sharding: core = b*4 + hg  (b = batch, hg = head-group of 4 heads).
Each core computes, for its batch b and its 4 heads:
    q^T = (Wq_g @ X_q^T + bq_g)      stored [256, 2048]  (head-dim on partitions)
    k^T likewise, v = X_v @ Wv_g^T + bv_g stored [2048, 4, 65] (ones col appended)
    S^T[s_k, s_q] = k^T.T-contraction(d)  per head
    P^T = exp(S^T / 8)                (no max-subtract; scores are O(1) here)
    attnout^T[o, s_q] (+ rowsum row) = [v | 1].T @ P^T   accumulated over s_k
    attnout_norm^T = attnout^T * (1/rowsum)  (per head)
    outT_partial[m, s] = Wo_g^T-contraction(o) @ attnout_norm^T   [1024, 2048] f32
Host gathers: out[b] = sum_g outT_partial.T + bo.

All matmul inputs bf16 (PSUM accumulation f32).
"""

import numpy as np
import ml_dtypes

import concourse.bacc as bacc
import concourse.mybir as mybir
import concourse.tile as tile
from concourse.bass_utils import run_bass_kernel_spmd

BF16 = mybir.dt.bfloat16
F32 = mybir.dt.float32
AF = mybir.ActivationFunctionType
ALU = mybir.AluOpType

B, S, D = 2, 2048, 1024
H = 16
DK = 64
NCORES = 8
HG = 4  # head groups
HPG = 4  # heads per group
GO = HPG * DK  # 256 group output width

_NC = None


def _emit(nc, tc, io):
    xqT, xkT, xvT, wqT, wkT, wvT, woT, bqk, outT = (
        io["xqT"], io["xkT"], io["xvT"], io["wqT"], io["wkT"], io["wvT"],
        io["woT"], io["bqk"], io["outT"],
    )
    NIC = D // 128  # 8 contraction chunks of 128
    NSC = S // 128  # 16 s chunks of 128
    NSQ = S // 512  # 4 s chunks of 512

    with (
        tc.tile_pool(name="wp", bufs=1) as wp,
        tc.tile_pool(name="xp", bufs=1) as xp,
        tc.tile_pool(name="pp", bufs=1) as pp,
        tc.tile_pool(name="pt", bufs=6) as ptp,
        tc.tile_pool(name="rs", bufs=2) as rsp,
        tc.tile_pool(name="rb", bufs=2) as rbp,
        tc.tile_pool(name="st", bufs=2) as stp,
        tc.tile_pool(name="fo", bufs=8) as fop,
        tc.tile_pool(name="psA", bufs=4, space="PSUM") as psA,
        tc.tile_pool(name="psB", bufs=2, space="PSUM") as psB,
        tc.tile_pool(name="dr", bufs=4, space="DRAM") as drp,
    ):
        # ---- load weights + inputs (in consumption order) ----
        bqk_t = wp.tile([128, 4], F32, name="bqk", tag="bqk")
        nc.sync.dma_start(bqk_t[:], bqk[:])
        wk = []
        xk = []
        for i in range(NIC):
            t = wp.tile([128, GO], BF16, name=f"wk{i}", tag=f"wk{i}")
            nc.sync.dma_start(t[:], wkT[128 * i:128 * i + 128, :])
            wk.append(t)
            t = xp.tile([128, S], BF16, name=f"xk{i}", tag=f"xk{i}")
            nc.gpsimd.dma_start(t[:], xkT[128 * i:128 * i + 128, :])
            xk.append(t)
        wq = []
        xq = []
        for i in range(NIC):
            t = wp.tile([128, GO], BF16, name=f"wq{i}", tag=f"wq{i}")
            nc.sync.dma_start(t[:], wqT[128 * i:128 * i + 128, :])
            wq.append(t)
            t = xp.tile([128, S], BF16, name=f"xq{i}", tag=f"xq{i}")
            nc.gpsimd.dma_start(t[:], xqT[128 * i:128 * i + 128, :])
            xq.append(t)
        wv = []
        xv = []
        for i in range(NIC):
            t = wp.tile([128, GO], BF16, name=f"wv{i}", tag=f"wv{i}")
            nc.sync.dma_start(t[:], wvT[128 * i:128 * i + 128, :])
            wv.append(t)
            t = xp.tile([128, S], BF16, name=f"xv{i}", tag=f"xv{i}")
            nc.gpsimd.dma_start(t[:], xvT[128 * i:128 * i + 128, :])
            xv.append(t)
        wv_b = wp.tile([1, GO], BF16, name="wvb", tag="wvb")
        nc.sync.dma_start(wv_b[:], wvT[D:D + 1, :])
        xv_ones = xp.tile([1, S], BF16, name="xvo", tag="xvo")
        nc.sync.dma_start(xv_ones[:], xvT[D:D + 1, :])
        wo = []
        for oc in range(2):
            t = wp.tile([128, D], BF16, name=f"wo{oc}", tag=f"wo{oc}")
            nc.sync.dma_start(t[:], woT[128 * oc:128 * oc + 128, :])
            wo.append(t)

        # ---- k^T / q^T projections (ic outer: accumulate as input DMAs
        # land; 4 PSUM accumulators per oc pass) ----
        def proj_T(out_tiles, w_tiles, x_tiles, bias_col0):
            for oc in range(2):
                accs = [psA.tile([128, 512], F32, name="acc", tag="acc")
                        for _ in range(NSQ)]
                for ic in range(NIC):
                    for sc in range(NSQ):
                        nc.tensor.matmul(
                            accs[sc][:],
                            w_tiles[ic][:, 128 * oc:128 * oc + 128],
                            x_tiles[ic][:, 512 * sc:512 * sc + 512],
                            start=(ic == 0),
                            stop=(ic == NIC - 1),
                        )
                for sc in range(NSQ):
                    nc.vector.tensor_scalar(
                        out_tiles[oc][:, 512 * sc:512 * sc + 512], accs[sc][:],
                        bqk_t[:, bias_col0 + oc:bias_col0 + oc + 1], None,
                        op0=ALU.add,
                    )

        kT = [pp.tile([128, S], BF16, name=f"kT{oc}", tag=f"kT{oc}") for oc in range(2)]
        proj_T(kT, wk, xk, 2)

        # ---- q^T projection ----
        qT = [pp.tile([128, S], BF16, name=f"qT{oc}", tag=f"qT{oc}") for oc in range(2)]
        proj_T(qT, wq, xq, 0)

        # ---- v projection: v[sc] [128, 4, 65] (ones col at 64) ----
        v = [pp.tile([128, HPG, DK + 1], BF16, name=f"v{sc}", tag=f"v{sc}") for sc in range(NSC)]
        for sc in range(NSC):
            nc.vector.memset(v[sc][:, :, DK:DK + 1], 1.0)
        for scg in range(NSC // NSQ):
            accs = [psA.tile([128, 512], F32, name="acc", tag="acc")
                    for _ in range(NSQ)]
            for ic in range(NIC):
                for j in range(NSQ):
                    sc = scg * NSQ + j
                    nc.tensor.matmul(
                        accs[j][:, 0:GO],
                        xv[ic][:, 128 * sc:128 * sc + 128],
                        wv[ic][:],
                        start=(ic == 0),
                        stop=False,
                    )
            for j in range(NSQ):
                sc = scg * NSQ + j
                nc.tensor.matmul(
                    accs[j][:, 0:GO],
                    xv_ones[:, 128 * sc:128 * sc + 128],
                    wv_b[:],
                    start=False,
                    stop=True,
                )
                nc.scalar.copy(
                    v[sc][:, :, 0:DK],
                    accs[j][:, 0:GO].rearrange("p (h d) -> p h d", d=DK),
                )

        # ---- attention (1-step software pipeline: scores/exp run one step
        # ahead of the PV matmuls; psB double-buffered per head-pair) ----
        attnT = [pp.tile([128, S], BF16, name=f"at{oc}", tag=f"at{oc}") for oc in range(2)]
        pvacc = {}  # sqc -> 4 PSUM accum tiles

        def normalize(c):
            # Drain the 4 PSUM accumulators to SBUF first (releases the PSUM
            # tiles for reuse); normalization then runs off the critical
            # path from the SBUF copies.  The last chunk computes 1/rowsum
            # as exp(-ln(x)) on the (then idle) scalar engine instead of the
            # slow DVE reciprocal, shortening the kernel tail.
            atcs = []
            for h in range(HPG):
                atc = rsp.tile([DK + 1, 512], F32, name="atc", tag="atc", bufs=5)
                nc.vector.tensor_copy(atc[:], pvacc[c][h][0:DK + 1, :])
                atcs.append(atc)
            for h in range(HPG):
                atc = atcs[h]
                rd_ = drp.tile([1, 512], F32, name="rd", tag="rd")
                if c == NSQ - 1:
                    # tail: 1/rowsum as exp(-ln) on the then-idle scalar engine
                    rs_ = rsp.tile([DK + 1, 512], F32, name="rs", tag="rs")
                    ln_ = rsp.tile([DK + 1, 512], F32, name="ln", tag="ln")
                    nc.scalar.activation(ln_[DK:DK + 1, :], atc[DK:DK + 1, :],
                                         AF.Ln)
                    nc.scalar.activation(rs_[DK:DK + 1, :], ln_[DK:DK + 1, :],
                                         AF.Exp, scale=-1.0)
                    nc.sync.dma_start(rd_[:], rs_[DK:DK + 1, :])
                else:
                    # mid-chunks: bounce the rowsum through DRAM reshaped to
                    # [128, 4] so the DVE reciprocal runs 128-lane-parallel
                    # (~0.2us instead of 3.3us on a single lane)
                    rw_ = drp.tile([1, 512], F32, name="rw", tag="rw")
                    nc.sync.dma_start(rw_[:], atc[DK:DK + 1, :])
                    rq_ = rsp.tile([128, 4], F32, name="rq", tag="rq", bufs=4)
                    nc.sync.dma_start(
                        rq_[:], rw_.rearrange("a (p j) -> (a p) j", p=128))
                    rr_ = rsp.tile([128, 4], F32, name="rr", tag="rr", bufs=4)
                    nc.vector.reciprocal(rr_[:], rq_[:])
                    nc.sync.dma_start(
                        rd_.rearrange("a (p j) -> (a p) j", p=128), rr_[:])
                rb_ = rbp.tile([DK, 512], F32, name="rb", tag="rb")
                nc.gpsimd.dma_start(rb_[:], rd_.to_broadcast([DK, 512]))
                st_ = stp.tile([DK, 512], BF16, name="st", tag="st")
                nc.vector.tensor_mul(st_[:], atc[0:DK, :], rb_[:])
                nc.gpsimd.dma_start(
                    attnT[h // 2][64 * (h % 2):64 * (h % 2) + 64,
                                  512 * c:512 * c + 512],
                    st_[:],
                )

        def emit_fproj(sc):
            # Output projection for s-chunk sc (runs at the very end; F(0..2)
            # overlap the last chunk's normalization chain).  Later chunks
            # drain their PSUM through the scalar engine to split the copy
            # load across both PSUM-capable engines.
            for mc in range(D // 128):
                fac = psA.tile([128, 512], F32, name="acc", tag="acc")
                for oc in range(2):
                    nc.tensor.matmul(
                        fac[:],
                        wo[oc][:, 128 * mc:128 * mc + 128],
                        attnT[oc][:, 512 * sc:512 * sc + 512],
                        start=(oc == 0),
                        stop=(oc == 1),
                    )
                fo_ = fop.tile([128, 512], BF16, name="fo", tag="fo")
                if sc >= 2:
                    nc.scalar.copy(fo_[:], fac[:])
                else:
                    nc.vector.tensor_copy(fo_[:], fac[:])
                eng = nc.sync if mc % 2 == 0 else nc.gpsimd
                eng.dma_start(
                    outT[128 * mc:128 * mc + 128, 512 * sc:512 * sc + 512],
                    fo_[:],
                )

        def emit_pv(prev):
            pTs, c, k = prev
            if k == 0:
                pvacc[c] = [
                    psA.tile([128, 512], F32, name="acc", tag="acc")
                    for _ in range(HPG)
                ]
            for h in range(HPG):
                nc.tensor.matmul(
                    pvacc[c][h][0:DK + 1, :],
                    v[k][:, h, :],
                    pTs[h // 2][:, 512 * (h % 2):512 * (h % 2) + 512],
                    start=(k == 0),
                    stop=(k == NSC - 1),
                )
            if k == NSC - 1:
                normalize(c)

        prev = None
        for sqc in range(NSQ):
            for skc in range(NSC):
                pTs = []
                for hp in range(2):
                    ps_ = psB.tile([128, 1024], F32, name="sc", tag="sc")
                    for sub in range(2):
                        nc.tensor.matmul(
                            ps_[:, 512 * sub:512 * sub + 512],
                            kT[hp][64 * sub:64 * sub + 64,
                                   128 * skc:128 * skc + 128],
                            qT[hp][64 * sub:64 * sub + 64,
                                   512 * sqc:512 * sqc + 512],
                            start=True,
                            stop=True,
                            tile_position=(64 * sub, 0),
                        )
                    pT_ = ptp.tile([128, 1024], BF16, name="pT", tag="pT")
                    nc.scalar.activation(pT_[:], ps_[:], AF.Exp, scale=0.125)
                    pTs.append(pT_)
                if prev is not None:
                    emit_pv(prev)
                prev = (pTs, sqc, skc)
        emit_pv(prev)
        for sc in range(NSQ):
            emit_fproj(sc)


def build_nc():
    nc = bacc.Bacc("TRN2", target_bir_lowering=False, debug=False,
                   num_devices=NCORES)
    io = {
        "xqT": nc.dram_tensor("xqT", [D, S], BF16, kind="ExternalInput").ap(),
        "xkT": nc.dram_tensor("xkT", [D, S], BF16, kind="ExternalInput").ap(),
        "xvT": nc.dram_tensor("xvT", [D + 1, S], BF16, kind="ExternalInput").ap(),
        "wqT": nc.dram_tensor("wqT", [D, GO], BF16, kind="ExternalInput").ap(),
        "wkT": nc.dram_tensor("wkT", [D, GO], BF16, kind="ExternalInput").ap(),
        "wvT": nc.dram_tensor("wvT", [D + 1, GO], BF16, kind="ExternalInput").ap(),
        "woT": nc.dram_tensor("woT", [GO, D], BF16, kind="ExternalInput").ap(),
        "bqk": nc.dram_tensor("bqk", [128, 4], F32, kind="ExternalInput").ap(),
        "outT": nc.dram_tensor("outT", [D, S], BF16, kind="ExternalOutput").ap(),
    }
    with tile.TileContext(nc) as tc:
        _emit(nc, tc, io)
    nc.compile()
    return nc


def get_nc():
    global _NC
    if _NC is None:
        _NC = build_nc()
    return _NC


def shard_inputs(Q, K, V, Wq, bq, Wk, bk, Wv, bv, Wo, bo):
    bf = ml_dtypes.bfloat16
    ones = np.ones((1, S), np.float32)
    in_maps = []
    for core in range(NCORES):
        b, hg = core // HG, core % HG
        rows = slice(GO * hg, GO * hg + GO)
        bq_g, bk_g, bv_g = bq[rows], bk[rows], bv[rows]
        bqk_t = np.stack(
            [bq_g[0:128], bq_g[128:256], bk_g[0:128], bk_g[128:256]], axis=1
        ).astype(np.float32)
        in_maps.append({
            "xqT": np.ascontiguousarray(Q[b].T).astype(bf),
            "xkT": np.ascontiguousarray(K[b].T).astype(bf),
            "xvT": np.concatenate([V[b].T, ones], 0).astype(bf),
            "wqT": np.ascontiguousarray(Wq[rows].T).astype(bf),
            "wkT": np.ascontiguousarray(Wk[rows].T).astype(bf),
            "wvT": np.concatenate([Wv[rows].T, bv_g[None, :]], 0).astype(bf),
            "woT": np.ascontiguousarray(Wo[:, rows].T).astype(bf),
            "bqk": bqk_t,
        })
    return in_maps


def kernel(**inputs):
    args = {k: np.asarray(v) for k, v in inputs.items()}
    nc = get_nc()
    in_maps = shard_inputs(
        args["Q"], args["K"], args["V"], args["Wq"], args["bq"], args["Wk"],
        args["bk"], args["Wv"], args["bv"], args["Wo"], args["bo"],
    )
    res = run_bass_kernel_spmd(nc, in_maps, list(range(NCORES)))
    out = np.zeros((B, S, D), np.float32)
    for core in range(NCORES):
        out[core // HG] += res.results[core]["outT"].astype(np.float32).T
    out += args["bo"].astype(np.float32)
    return out
